# revision 13
# baseline (speedup 1.0000x reference)
"""Trainium2 Bass kernel for nn_CapsuleLayer (dynamic routing capsule layer).

Sharding: the 1152 input capsules (i) are split across 8 cores (144 each);
the full batch B=128 lives on SBUF partitions. Routing state (c, p) stays
local to each core's i-shard; the per-iteration s partial sums are combined
with 3 small AllReduces ([128,160] f32). u_hat is never materialized — both
big contractions are rewritten through W:
  s[b,j,d]       = sum_{i,k} p[b,j,i] x[b,i,k] W[j,i,d,k]        (PE)
  c_delta[b,j,i] = sum_k x[b,i,k] m[b,j,i,k],   m = sum_d v[b,j,d] W[j,i,d,k]
(m via PE d-contraction in float32r; the x-multiply fuses with PSUM evac.)

v2 notes vs the first version:
  - all transposes on the PE (no DMA-transpose; they head-of-line blocked
    the sync sequencer), evacuations balanced between DVE and Act
  - (j,d)-major s/v layout end-to-end: no strided rearranges, the final v
    is DMA-ready, and v^T is built with 2 transposes instead of 10
  - squash uses 1/sqrt(z) = exp(-0.5*ln z) so exp/ln/copy/square all live
    in one activation table (no ACT_TABLE_LOAD thrash)
  - w^T (d-major W for the m-matmuls) loaded with 2 bulk DMAs up front
  - p = e * (1/sigma) folded before the per-j transposes (drops the
    separate rin transpose + x rescale)
"""

import sys

if "/opt/trn_rl_repo" not in sys.path:
    sys.path.insert(0, "/opt/trn_rl_repo")

import contextlib

import numpy as np

import concourse.bass as bass  # noqa: F401
import concourse.tile as tile
from concourse import bacc, mybir
from concourse.bass_utils import run_bass_kernel_spmd
from concourse.masks import make_identity

f32 = mybir.dt.float32
f32r = mybir.dt.float32r
bf16 = mybir.dt.bfloat16
AL = mybir.AluOpType
AF = mybir.ActivationFunctionType

B = 128          # batch (on partitions)
NJ = 10          # output capsules
DO = 16          # output capsule dim
DI = 8           # input capsule dim
NI = 1152        # input capsules (global)
ROUTINGS = 3
EPS = 1e-7


def build_kernel(n_cores=8, debug=False, repeat=1, single=False, ablate=()):
    ni_l = NI // n_cores
    chunks = []
    o = 0
    while o < ni_l:
        chunks.append((o, min(128, ni_l - o)))
        o += 128

    nc = bacc.Bacc("TRN2", target_bir_lowering=False, debug=False,
                   num_devices=1 if single else n_cores)
    x_d = nc.dram_tensor("x", [B, ni_l, DI], f32, kind="ExternalInput")
    w_d = nc.dram_tensor("w", [NJ, ni_l, DO, DI], f32, kind="ExternalInput")
    out_d = nc.dram_tensor("out", [B, NJ, DO], f32, kind="ExternalOutput")
    dbg = {}
    if debug:
        dbg["c"] = nc.dram_tensor("dbg_c", [B, NJ, ni_l], f32, kind="ExternalOutput")
        dbg["p"] = nc.dram_tensor("dbg_p", [B, NJ, ni_l], f32, kind="ExternalOutput")
        dbg["s0"] = nc.dram_tensor("dbg_s0", [B, NJ, DO], f32, kind="ExternalOutput")

    with tile.TileContext(nc) as tc:
        for _rep in range(repeat):
            _body(nc, tc, x_d, w_d, out_d, dbg if _rep == repeat - 1 else {},
                  ni_l, chunks, n_cores, single, ablate)
    nc.compile()
    return nc


def _body(nc, tc, x_d, w_d, out_d, dbg, ni_l, chunks, n_cores, single=False, ablate=()):
    ctx = contextlib.ExitStack()
    with ctx:
        sb = ctx.enter_context(tc.tile_pool(name="sb", bufs=1))
        sc = ctx.enter_context(tc.tile_pool(name="scratch", bufs=3))
        ps = ctx.enter_context(tc.tile_pool(name="ps", bufs=2, space="PSUM"))
        ps_acc = ctx.enter_context(tc.tile_pool(name="ps_acc", bufs=2, space="PSUM"))
        dram = ctx.enter_context(tc.tile_pool(name="dram", bufs=1, space="DRAM"))

        n_sl = (ni_l * DI) // 384        # 384-wide m-matmul slices
        PSPAD = [B, n_sl * 512]          # psum tiles padded to a common 3-bank shape

        def pst(dtype):
            # rotating psum tile for transposes ([<=128, B]) — shared tag
            pad = PSPAD if dtype == f32 else [B, 2 * n_sl * 512]
            return ps.tile([128, B], dtype, tag="tp", name="pt", padded_shape=pad)

        # ---------------- Phase 0: input DMAs (no deps; queue early) --------
        x_f = sc.tile([B, ni_l * DI], f32, tag="xload")
        nc.sync.dma_start(out=x_f, in_=x_d.ap().rearrange("b i k -> b (i k)"))

        w_f = []
        for c0, cn in chunks:
            wf = sc.tile([cn, NJ, DO, DI], f32, tag=f"wload{c0}")
            nc.sync.dma_start(
                out=wf, in_=w_d.ap()[:, c0:c0 + cn, :, :].rearrange("j i d k -> i j d k"))
            w_f.append(wf)

        # d-major W for the m-matmuls: [d, j, i, k] f32 (used as f32r), 2 bulk DMAs
        w_dT = sb.tile([DO, NJ, ni_l, DI], f32r)
        for jh in range(2):
            js = slice(jh * NJ // 2, (jh + 1) * NJ // 2)
            nc.sync.dma_start(
                out=w_dT[:, js, :, :],
                in_=w_d.ap()[js].rearrange("j i d k -> d j i k").bitcast(f32r))

        ident = sb.tile([128, 128], bf16)
        make_identity(nc, ident)
        ident_f = sb.tile([128, 128], f32)
        make_identity(nc, ident_f)

        # ---------------- casts ----------------
        x_bf = sb.tile([B, ni_l, DI], bf16)
        nc.scalar.copy(out=x_bf.rearrange("b i k -> b (i k)"), in_=x_f)
        w_bf = []
        for ci, (c0, cn) in enumerate(chunks):
            wb = sb.tile([cn, NJ, DO, DI], bf16, tag=f"wbf{c0}")
            nc.scalar.copy(out=wb, in_=w_f[ci])
            w_bf.append(wb)

        # x_P: [(i)ch, k, b] bf16 via PE transposes of k-slices
        x_P = [sb.tile([cn, DI, B], bf16, tag=f"xP{c0}", name=f"xP{c0}")
               for c0, cn in chunks]
        for ci, (c0, cn) in enumerate(chunks):
            for k in range(DI):
                pt = pst(bf16)
                nc.tensor.transpose(pt[:cn, :], x_bf[:, c0:c0 + cn, k], ident)
                eng = nc.vector if k % 2 == 0 else nc.scalar
                if eng is nc.vector:
                    eng.tensor_copy(out=x_P[ci][:, k, :], in_=pt[:cn, :])
                else:
                    eng.copy(out=x_P[ci][:, k, :], in_=pt[:cn, :])

        # ---------------- r0: s0 = (1/NJ) * sum_ik x W ----------------
        ps_s0 = ps_acc.tile([B, NJ, DO], f32, tag="smm")
        nmm = len(chunks) * DI
        imm = 0
        for ci, (c0, cn) in enumerate(chunks):
            for k in range(DI):
                nc.tensor.matmul(
                    ps_s0.rearrange("b j d -> b (j d)"),
                    lhsT=x_P[ci][:, k, :],
                    rhs=w_bf[ci][:, :, :, k].rearrange("i j d -> i (j d)"),
                    start=(imm == 0), stop=(imm == nmm - 1),
                )
                imm += 1
        s_part = sb.tile([B, NJ, DO], f32)
        nc.scalar.mul(out=s_part.rearrange("b j d -> b (j d)"),
                      in_=ps_s0.rearrange("b j d -> b (j d)"), mul=1.0 / NJ)
        if dbg:
            nc.sync.dma_start(out=dbg["s0"].ap(), in_=s_part)

        # persistent state tiles
        c_t = sb.tile([B, NJ, ni_l], f32)        # routing logits (j, i)
        s_full = sb.tile([B, NJ, DO], f32)       # all-reduced s
        v_f = sb.tile([B, NJ, DO], f32)          # squashed v, (j, d)-major
        v_T = sb.tile([DO, NJ, B], f32r)         # v transposed [d, j, b]
        e_bf = sb.tile([B, NJ, ni_l], bf16)      # exp(c)
        p_bf = sb.tile([B, NJ, ni_l], bf16)      # softmax p = e / sigma
        es5 = sb.tile([B, 5, ni_l], bf16)        # sigma tree scratch
        ssum = sb.tile([B, ni_l], bf16)          # sum_j exp(c)
        rin = sb.tile([B, ni_l], bf16)           # 1/ssum
        pT = [[sb.tile([cn, B], bf16, tag=f"pT{j}_{c0}", name=f"pT{j}_{c0}")
               for c0, cn in chunks] for j in range(NJ)]
        t_all = sb.tile([B, NJ, ni_l, DI], bf16)  # m * x scratch (all j)
        sq = sb.tile([B, NJ], f32)
        fac = sb.tile([B, NJ], f32)
        eps_t = sb.tile([B, 1], f32)
        nc.vector.memset(eps_t, EPS)
        one_t = sb.tile([B, 1], f32)
        nc.vector.memset(one_t, 1.0)

        ar_in = dram.tile([B, DO * NJ], f32)
        ar_out = dram.tile([B, DO * NJ], f32)

        def allreduce_s():
            nc.sync.dma_start(out=ar_in, in_=s_part.rearrange("b j d -> b (j d)"))
            if single:
                nc.sync.dma_start(out=ar_out, in_=ar_in)
            else:
                nc.gpsimd.collective_compute(
                    "AllReduce", AL.add,
                    ins=[ar_in.opt()], outs=[ar_out.opt()],
                    replica_groups=[list(range(n_cores))],
                )
            nc.sync.dma_start(out=s_full.rearrange("b j d -> b (j d)"), in_=ar_out)

        def squash(last):
            # sq = sum_d s^2 ; v = s * sq/(1+sq)/sqrt(sq+eps)
            # 1/sqrt(z) computed as exp(-0.5*ln z) to stay in one act table
            ssq = sc.tile([B, NJ, DO], f32, tag="sqt")
            nc.scalar.square(out=ssq.rearrange("b j d -> b (j d)"),
                             in_=s_full.rearrange("b j d -> b (j d)"))
            nc.vector.tensor_reduce(out=sq, in_=ssq,
                                    axis=mybir.AxisListType.X, op=AL.add)
            lnv = sc.tile([B, NJ], f32, tag="lnv")
            nc.scalar.activation(out=lnv, in_=sq, func=AF.Ln, bias=eps_t)
            rsq = sc.tile([B, NJ], f32, tag="rsq")
            nc.scalar.activation(out=rsq, in_=lnv, func=AF.Exp, scale=-0.5)
            onep = sc.tile([B, NJ], f32, tag="onep")
            nc.scalar.activation(out=onep, in_=sq, func=AF.Identity, bias=one_t)
            den = sc.tile([B, NJ], f32, tag="den")
            nc.vector.reciprocal(out=den, in_=onep)
            nc.vector.tensor_mul(out=fac, in0=sq, in1=rsq)
            nc.vector.tensor_mul(out=fac, in0=fac, in1=den)
            nc.vector.tensor_mul(
                out=v_f, in0=s_full,
                in1=fac.unsqueeze(2).broadcast_to([B, NJ, DO]))
            if not last:
                # v_T[d, j, b] via per-j PE transposes (base-partition-0 slices
                # for the m-matmul lhsT; PE requires operand base 0/32/64)
                for j in range(NJ):
                    ptv = pst(f32)
                    nc.tensor.transpose(ptv[:DO, :], v_f[:, j, :], ident_f)
                    if j % 2 == 0:
                        nc.scalar.copy(out=v_T[:, j, :], in_=ptv[:DO, :])
                    else:
                        nc.vector.tensor_copy(out=v_T[:, j, :], in_=ptv[:DO, :])

        w_dik = w_dT.rearrange("d j i k -> d j (i k)")
        x_ik = x_bf.rearrange("b i k -> b (i k)")
        x_3s = x_ik.rearrange("b (s e) -> b s e", s=n_sl)

        # j's whose m goes through an Act bf16 evac (then a cheap 4x DVE
        # multiply) vs. a direct DVE multiply from PSUM
        ACT_PATH = (0, 1, 2, 3, 5, 6, 7, 8)

        def c_update(first):
            # m_j = sum_d v[b,j,d] W[j,:,d,:] ; c += sum_k x*m  (k-tree)
            if "cupd" in ablate:
                if first:
                    nc.vector.memset(c_t, 0.0)
                return
            for j in range(NJ):
                pm3 = ps.tile([B, n_sl, 512], f32, tag="tp", name="pm3")
                for sl in range(n_sl):
                    nc.tensor.matmul(
                        pm3[:, sl, 0:384],
                        lhsT=v_T[:, j, :],
                        rhs=w_dik[:, j, 384 * sl:384 * (sl + 1)],
                        start=True, stop=True,
                    )
                tj3 = t_all[:, j, :, :].rearrange("b i k -> b (i k)").rearrange(
                    "b (s e) -> b s e", s=n_sl)
                if j in ACT_PATH:
                    m_bf = sc.tile([B, n_sl, 384], bf16, tag="m_bf", name="m_bf")
                    nc.scalar.copy(out=m_bf, in_=pm3[:, :, 0:384])
                    nc.vector.tensor_tensor(out=tj3, in0=m_bf, in1=x_3s, op=AL.mult)
                else:
                    nc.vector.tensor_tensor(out=tj3, in0=pm3[:, :, 0:384],
                                            in1=x_3s, op=AL.mult)
            if "c_mul" in ablate:
                nc.vector.memset(c_t, 0.0)
                return
            # k-tree: 8 -> 4 -> 2 -> c
            nc.vector.tensor_tensor(out=t_all[:, :, :, 0:4], in0=t_all[:, :, :, 0:4],
                                    in1=t_all[:, :, :, 4:8], op=AL.add)
            nc.vector.tensor_tensor(out=t_all[:, :, :, 0:2], in0=t_all[:, :, :, 0:2],
                                    in1=t_all[:, :, :, 2:4], op=AL.add)
            if first:
                nc.vector.tensor_tensor(out=c_t, in0=t_all[:, :, :, 0],
                                        in1=t_all[:, :, :, 1], op=AL.add)
            else:
                tmp = sc.tile([B, NJ, ni_l], bf16, tag="ctmp")
                nc.vector.tensor_tensor(out=tmp, in0=t_all[:, :, :, 0],
                                        in1=t_all[:, :, :, 1], op=AL.add)
                nc.vector.tensor_tensor(out=c_t, in0=c_t, in1=tmp, op=AL.add)

        def softmax_and_s():
            if "smax" in ablate:
                return
            # e = exp(c); sigma = sum_j e (tree); p = e / sigma
            for jh in range(2):
                jsl = slice(jh * NJ // 2, (jh + 1) * NJ // 2)
                nc.scalar.activation(out=e_bf[:, jsl, :], in_=c_t[:, jsl, :],
                                     func=AF.Exp)
            nc.vector.tensor_tensor(out=es5, in0=e_bf[:, 0:5, :],
                                    in1=e_bf[:, 5:10, :], op=AL.add)
            nc.vector.tensor_tensor(out=es5[:, 0:2, :], in0=es5[:, 0:2, :],
                                    in1=es5[:, 2:4, :], op=AL.add)
            nc.vector.tensor_tensor(out=ssum, in0=es5[:, 0, :],
                                    in1=es5[:, 1, :], op=AL.add)
            nc.vector.tensor_tensor(out=ssum, in0=ssum,
                                    in1=es5[:, 4, :], op=AL.add)
            with nc.allow_low_precision(reason="softmax sigma/recip in bf16"):
                nc.vector.reciprocal(out=rin, in_=ssum)
            nc.vector.tensor_tensor(
                out=p_bf, in0=e_bf,
                in1=rin.unsqueeze(1).broadcast_to([B, NJ, ni_l]), op=AL.mult)
            # all p-transposes up front on the PE queue (2-deep psum rotation)
            for j in range(NJ):
                for ci, (c0, cn) in enumerate(chunks):
                    pt = pst(bf16)
                    nc.tensor.transpose(pt[:cn, :], p_bf[:, j, c0:c0 + cn], ident)
                    if ci == 0:
                        nc.scalar.copy(out=pT[j][ci], in_=pt[:cn, :])
                    else:
                        nc.vector.tensor_copy(out=pT[j][ci], in_=pt[:cn, :])
            if "s_tp" in ablate:
                return
            ps_s = ps_acc.tile([B, NJ, DO], f32, tag="smm", name="ps_s")
            for j in range(NJ):
                for ci, (c0, cn) in enumerate(chunks):
                    y = sc.tile([cn, DI, B], bf16, tag=f"y{ci}", name="y")
                    nc.vector.tensor_tensor(
                        out=y, in0=x_P[ci],
                        in1=pT[j][ci].unsqueeze(1).broadcast_to([cn, DI, B]),
                        op=AL.mult)
                    if "s_mm" in ablate:
                        continue
                    for k in range(DI):
                        nc.tensor.matmul(
                            ps_s[:, j, :],
                            lhsT=y[:, k, :],
                            rhs=w_bf[ci][:, j, :, k],
                            start=(ci == 0 and k == 0),
                            stop=(ci == len(chunks) - 1 and k == DI - 1),
                        )
            if "s_mm" not in ablate:
                nc.scalar.copy(out=s_part.rearrange("b j d -> b (j d)"),
                               in_=ps_s.rearrange("b j d -> b (j d)"))

        # ---------------- routing ----------------
        if {"s_tp", "s_mm", "smax"} & set(ablate):
            nc.vector.memset(s_part, 0.0)
        allreduce_s()          # r0 s
        squash(last=False)     # r0 v
        c_update(first=True)   # c1
        for r in range(1, ROUTINGS):
            last = (r == ROUTINGS - 1)
            softmax_and_s()
            allreduce_s()
            squash(last=last)
            if not last:
                c_update(first=False)
        if dbg:
            nc.sync.dma_start(out=dbg["c"].ap(), in_=c_t)
            p_f = sb.tile([B, NJ, ni_l], f32)
            nc.vector.tensor_copy(out=p_f, in_=p_bf)
            nc.sync.dma_start(out=dbg["p"].ap(), in_=p_f)

        nc.sync.dma_start(out=out_d.ap(), in_=v_f)


_NC_CACHE = {}


def kernel(inputs: np.ndarray, W: np.ndarray) -> np.ndarray:
    n_cores = 8
    ni_l = NI // n_cores
    if "nc" not in _NC_CACHE:
        _NC_CACHE["nc"] = build_kernel(n_cores=n_cores, debug=False)
    nc = _NC_CACHE["nc"]
    in_maps = []
    for r in range(n_cores):
        sl = slice(ni_l * r, ni_l * (r + 1))
        in_maps.append({
            "x": np.ascontiguousarray(inputs[:, sl, :], dtype=np.float32),
            "w": np.ascontiguousarray(W[:, sl, :, :], dtype=np.float32),
        })
    res = run_bass_kernel_spmd(nc, in_maps, core_ids=list(range(n_cores)))
    return res.results[0]["out"]
